# revision 1
# baseline (speedup 1.0000x reference)
"""Multi-head attention kernel for Trainium2, SPMD across 8 NeuronCores.

Problem: q,k,v [B=2, H=16, S=2048, D=64] f32;
  out = softmax(q @ k^T / sqrt(4)) @ v      (scale quirk: d_k = tensor RANK = 4)

Sharding: 32 (b,h) heads split 4-per-core across 8 cores; the forward pass is
fully data-parallel (no collectives).

v2 changes over the original (HW-calibrated via microbenchmarks; the axon
deployment has no NTFF hook, so per-engine rates were measured with
slope-method microbenches against the TimelineSim cost model — PE fp32r
0.81x model, ACT 0.83x, DVE 0.99x, GPSIMD 2.14x SLOWER, 256B-chunk strided
DMA 3.1x SLOWER):
  - DMA: the kernel-layout gather "(n p) d -> p n d" moves 256B chunks per
    descriptor and measured ~3.1x slower than modeled (~57GB/s/core). All
    input/output DMAs now use a PERMUTED sequence order s = n*1024 + p*8 + r
    ("(n p r) d -> p n r d"), which makes every DMA move 2KB-contiguous
    DRAM chunks per partition. Attention is permutation-invariant along t
    (softmax sums over all keys), and the q-permutation is undone by the
    output DMA's matching rearrange, so results are unchanged.
  - The Schraudolph fast-exp path wrote int32 via DVE then bounced through a
    GPSIMD tensor_copy to get f32r-typed bits; GPSIMD copies measured 2.1x
    slower than modeled (~1.9us per [128,512] tile), making Pool a ~119us
    serial lane. The bounce is gone: DVE writes straight into the fp32r tile
    through a dst bitcast (PE reads TF32-truncated bits; harmless vs the
    fast-exp's ~3% error).
  - Scores for the two packed heads now land in ONE two-bank PSUM tile
    [128, 1024], so each t-iter needs a single exp instruction (ACT fixed
    overhead is ~172+ cycles/instruction; halving instruction count saves
    ~25% ACT busy time).
  - PV matmuls run in bf16 (same 1 cycle/row PE rate as fp32r). This lets
    the fast-exp path write int16 bf16-bit-patterns via a dst bitcast --
    the BIR verifier rejects non-f32r-rounding producers for fp32r matmul
    inputs, which is what forced the GPSIMD bounce originally. bf16 P/V
    adds ~0.2-0.4% elementwise error, small vs the fast-exp's ~3%.
  - Per-head outputs are staged in SBUF across all 4 q-chunks and stored
    as one 512KB DMA per head (2KB chunks) at pair end.

v3/v4 changes (second optimization round):
  - QK matmuls are now FULL-HEIGHT: K^T is stored split-padded per head
    (kta = [ktA ; 0], ktb = [0 ; ktB], pads zeroed once at startup), with
    the packed Q^T as the shared 128-partition moving operand -- the zero
    rows annul the other head. Microbenches showed 64-row half-height
    stationaries cost ~2x per row on HW (320ns vs 173ns per 512-row
    matmul) while alternating full-height stationaries is free, so the
    same matmul cycles buy twice the throughput. (The original design
    assumed the two heads' half-height matmuls run concurrently on
    disjoint PE row groups; on this hardware they do not.)
  - V1 builds moved to the otherwise-idle GPSIMD engine (SBUF-to-SBUF
    only; GPSIMD cannot touch PSUM).
  - Head A's O-accumulator is double-buffered (the epilogue-transpose
    staging shares ps_t's bank via a same-tag tile, freeing one), so the
    next q-chunk's first PV matmul doesn't wait on the PSUM-release copy.

v5 changes (third round): the per-pair work is one global iteration
stream -- PV lags the scores by 2 iterations ACROSS q-chunk boundaries
(no pipeline drain/refill at the 3 interior boundaries), and the epilogue
is split: PSUM-release copies issue right after each chunk's last PV,
while the PE transposes+normalize run 4 iterations later (slot t=5 of the
next chunk) so they never block the next scores matmuls in PE program
order.

Measured (slope method over a 1-rep vs 25-rep NEFF, outlier-robust
anchored medians; see test.py): ~129-142us/invocation across runs (the
shared machine's quiet-state floor itself drifts a few hundred us per
call between runs) vs ~333-386us for the original; rel err (max/max)
~1.1e-2. Also probed: f32r-typed Q/K staging to get 1.5-cycle/row
transposes -- the required casting DMA only exists on the GPSIMD DGE
queue and costs more than the 3.4us of PE it saves. Note t(reps) is measurably
non-linear: a 25-vs-49-rep contrast reads ~167us/rep (larger fully
unrolled NEFFs run slower per rep -- instruction-fetch or DVFS effects),
so slope numbers are only comparable at matched rep counts. TimelineSim per-rep slope is 132us
with PE.ENGINE busy 121us/rep: tensor-engine-bound, within ~10us of the
model. Probed and ruled out: LoadStationary churn (free, even full-height
alternating), fp8/DoubleRow (error budget), QCHUNK=256 for more PSUM
elasticity (per-instruction overhead dominates). Tried and reverted:
deferring per-qc epilogue transposes into the next chunk's PE slack slots
(the deferred transposes stall on the PSUM-release copy in program order
ahead of the next scores matmuls; measured slower).

Per-core algorithm (flash-attention style, scores kept TRANSPOSED so the
probability tiles come out already in the orientation the P@V matmul needs):
  - Build paired Q^T, K^T [128, S] (partitions 0-63 head A's [d, s], 64-127
    head B's) via TensorE transposes whose free axis is (head, d): one
    [128,128] transpose per s-tile lands both heads at once. The transposes
    are emitted as work units interleaved into the PREVIOUS head-pair's main
    loop so they ride in PE slack cycles. The two heads' QK^T matmuls target
    disjoint PE row groups (tile_position from base partition) and run
    CONCURRENTLY, halving the K=64 score-matmul wall time.
  - For each q-chunk (512 q) and t-tile (128 t):
      S^T[t, (A q | B q)] = K^T_tile.T @ Q^T_chunk   (two matmuls into one
                                                      2-bank PSUM tile)
      P^T = exp(0.5 * S^T)          one [128,1024] instruction; most t-tiles
                                    on ScalarE Exp, a subset on VectorE
                                    Schraudolph fast-exp (s*EXPA+EXPB ->
                                    int32 bits reinterpreted as f32, ~3% max
                                    elementwise error) to balance engine load
      O^T[d+2, q] += V1_tile.T @ P^T_head  (V1 = [V | ones | ones]: row 64
                                            accumulates the softmax
                                            denominator for free)
    PV matmuls are software-pipelined one t-tile behind the scores matmuls.
  - Epilogue per q-chunk: transpose O^T back to [q, 66] via TensorE,
    multiply by reciprocal(denominator) on VectorE, DMA out (1KB chunks).

The big matmuls run in float32r (TF32-style fast fp32: 1 cycle/row vs 4 for
plain fp32; microbenched at full rate). fp32r ISA restrictions: even
innermost free counts, 8B-aligned dst offsets, dst start_partition 0 —
hence V1 padded to 66 columns.

No max-subtraction in the softmax: scaled scores are ~N(0, 4) with |s| < ~25
for these inputs, so exp stays in f32 range and softmax's scale invariance
cancels any constant bias.
"""

import numpy as np

B, H, S, D = 2, 16, 2048, 64
N_CORES = 8
HPC = (B * H) // N_CORES  # heads per core = 4
P = 128
RB = 8  # DRAM rows per partition chunk (2KB) in the permuted layout
NB = S // (P * RB)  # n blocks = 2
T_TILES = S // P  # 16
QCHUNK = 512
N_QCHUNKS = S // QCHUNK  # 4
VE = D + 2  # V1 columns: 64 data + 1 ones (denominator) + 1 pad
SCALE = 0.5  # 1/sqrt(d_k) with d_k = k.ndim = 4 (faithful to reference)
# Schraudolph fast-exp constants, bf16 flavor: exp(SCALE*s) ~=
# bitcast_bf16(int16(s*EXPA16 + EXPB16)). The PV matmuls run in bf16 (same
# 1 cycle/row PE rate as fp32r) so the int16 bit-pattern write needs no
# f32r-rounding producer, which the BIR verifier enforces for fp32r.
EXPA16 = 0.5 * 128.0 * 1.4426950408889634
EXPB16 = 16256.0 - 5.6
# t-tile indices (mod 16) computed with the fast-exp on VectorE instead of
# ScalarE Exp. 4/16 of tiles, spread mid-chunk and kept away from t=13-15:
# the chunk-end window is when the epilogue's PSUM-release copies and
# normalize muls queue on VectorE, and a fast-exp burst there delays the
# accumulator release that gates the next chunk's PV matmuls. ScalarE has
# the slack to absorb the difference (~85us vs PE's ~105us real).
SCH = (2, 5, 8, 11)

_CACHE = {}


def _build_nc(reps=1, sch=SCH):
    from contextlib import ExitStack

    import concourse.bacc as bacc
    import concourse.mybir as mybir
    import concourse.tile as tile
    from concourse.masks import make_identity

    fp32 = mybir.dt.float32
    fp32r = mybir.dt.float32r
    bf16 = mybir.dt.bfloat16
    i16 = mybir.dt.int16
    Exp = mybir.ActivationFunctionType.Exp

    nc = bacc.Bacc()
    q_ext = nc.declare_dram_parameter("q", [HPC, S, D], fp32, isOutput=False)
    k_ext = nc.declare_dram_parameter("k", [HPC, S, D], fp32, isOutput=False)
    v_ext = nc.declare_dram_parameter("v", [HPC, S, D], fp32, isOutput=False)
    out_ext = nc.declare_dram_parameter("out", [HPC, S, D], fp32, isOutput=True)

    with ExitStack() as ctx:
        tc = ctx.enter_context(tile.TileContext(nc))
        consts = ctx.enter_context(tc.tile_pool(name="consts", bufs=1))
        identity = consts.tile([P, P], fp32)
        make_identity(nc, identity)
        # dummy exp: forces the ACT exp table-set DMA (~2.7us) to happen here,
        # overlapped with the input DMA lead-in, not at the first real exp.
        actwarm = consts.tile([P, 2], fp32)
        nc.scalar.activation(out=actwarm, in_=identity[:, 0:2], func=Exp, scale=1.0)
        # one-time zero source for the K^T pad halves (gpsimd memset cannot
        # write f32r; a DVE tensor_copy fp32->f32r is a rounding producer)
        zsrc = consts.tile([D, S], fp32, tag="zsrc", name="zsrc")
        nc.gpsimd.memset(zsrc, 0.0)

        nat = ctx.enter_context(tc.tile_pool(name="nat", bufs=2))
        vpool = ctx.enter_context(tc.tile_pool(name="vpool", bufs=2))
        qkt = ctx.enter_context(tc.tile_pool(name="qkt", bufs=2))
        ktp = ctx.enter_context(tc.tile_pool(name="ktp", bufs=2))
        ptp = ctx.enter_context(tc.tile_pool(name="ptp", bufs=10))
        otp = ctx.enter_context(tc.tile_pool(name="otp", bufs=4))
        op = ctx.enter_context(tc.tile_pool(name="op", bufs=2))
        rp = ctx.enter_context(tc.tile_pool(name="rp", bufs=4))
        # PSUM budget (8 banks of 2KB/partition):
        #   scores [128,1024] (2 banks) x2 bufs = 4 banks, O-acc A/B = 2 banks,
        #   qk-transpose staging 1 bank, epilogue-transpose staging 1 bank.
        ps_s = ctx.enter_context(tc.tile_pool(name="ps_s", bufs=2, space="PSUM"))
        ps_oA = ctx.enter_context(tc.tile_pool(name="ps_oA", bufs=2, space="PSUM"))
        ps_oB = ctx.enter_context(tc.tile_pool(name="ps_oB", bufs=1, space="PSUM"))
        ps_t = ctx.enter_context(tc.tile_pool(name="ps_t", bufs=1, space="PSUM"))

        def prep_pair(hA, hB):
            """Emit DMA loads + V1 builds; return (state, transpose work units).

            The transpose units are emitted by the caller interleaved into the
            previous pair's ACT-bound main loop so the PE does them in slack
            cycles instead of a serial phase where ScalarE would idle.
            """
            # [p, n, r, head, d]: permuted layout; DRAM row s = n*1024+p*8+r.
            # head next-to-last so a (n,r) slice exposes a contiguous (h d)
            # 128-wide free dim for the paired transpose; DRAM-side reads
            # stay sequential within each partition's 2KB block.
            qn = nat.tile([P, NB, RB, 2, D], fp32, tag="qn", name="qn")
            kn = nat.tile([P, NB, RB, 2, D], fp32, tag="kn", name="kn")
            vn = nat.tile([P, NB, RB, 2, D], fp32, tag="vn", name="vn")
            # K blocks land before Q blocks so the transpose units (ordered K
            # then Q per group) are fed in emission order; V gates only the
            # first PV matmul, so it goes last.
            for ext, dst in ((k_ext, kn), (q_ext, qn), (v_ext, vn)):
                for i, hh in enumerate((hA, hB)):
                    src = ext[hh].rearrange("(n p r) d -> p n r d", p=P, r=RB)
                    for z in range(NB):
                        nc.sync.dma_start(out=dst[:, z, :, i, :], in_=src[:, z])
            # V1 = [V | ones | ones] per head, built on VectorE (DVE) so the
            # PV matmul only waits on {DVE, ACT} producers.
            v1s = []
            for i in range(2):
                v1 = vpool.tile([P, T_TILES, VE], bf16, tag=f"v1{i}", name="v1")
                nc.gpsimd.tensor_copy(
                    out=v1[:, :, 0:D],
                    in_=vn[:, :, :, i, :].rearrange("p n r d -> p (n r) d"),
                )
                nc.gpsimd.tensor_scalar(
                    out=v1[:, :, D:VE],
                    in0=vn[:, :, :, i, 0:2].rearrange("p n r c -> p (n r) c"),
                    scalar1=0.0,
                    scalar2=1.0,
                    op0=mybir.AluOpType.mult,
                    op1=mybir.AluOpType.add,
                )
                v1s.append(v1)
            # Packed Q^T [128, S]: partitions 0-63 head A's [d, s], 64-127
            # head B's, built with ONE [128,128] transpose per s-tile whose
            # free axis is (head, d). t-tile tau = n*8+r; columns enumerate p.
            # K^T is stored SPLIT-PADDED: kta = [ktA ; 0], ktb = [0 ; ktB],
            # so each head's QK matmul is a FULL-HEIGHT 128-contraction with
            # the shared packed qt as moving operand (the zero rows annul the
            # other head). Half-height (64-row) stationaries measured ~2x
            # cost per row on HW (320ns vs 173ns per 512-row matmul);
            # alternating full-height stationaries is free.
            qt = qkt.tile([P, S], fp32r, tag="qt", name="qt")
            kta = ktp.tile([P, S], fp32r, tag="kta", name="kta")
            ktb = ktp.tile([P, S], fp32r, tag="ktb", name="ktb")
            kts = (kta, ktb)

            def unit(g, srcn, dsts):
                def emit():
                    tp = ps_t.tile([P, 4, P], fp32, tag="qk_t", name="tp")
                    for j in range(4):
                        tau = g * 4 + j
                        nc.tensor.transpose(
                            tp[:, j],
                            srcn[:, tau // RB, tau % RB].rearrange(
                                "p h d -> p (h d)"
                            ),
                            identity,
                        )
                    tpf = tp.rearrange("p a b -> p (a b)")
                    for dst, p0, p1 in dsts:
                        nc.vector.tensor_copy(
                            out=dst[p0:p1, g * 512 : (g + 1) * 512],
                            in_=tpf[p0:p1],
                        )
                return emit

            units = [
                unit(g, srcn, dsts)
                for srcn, dsts in (
                    (kn, ((kta, 0, D), (ktb, D, P))),
                    (qn, ((qt, 0, P),)),
                )
                for g in range(4)
            ]
            return (qt, kts, v1s), units

        for _ in range(2):
            zka = ktp.tile([P, S], fp32r, tag="kta", name="kta")
            zkb = ktp.tile([P, S], fp32r, tag="ktb", name="ktb")
            nc.vector.tensor_copy(out=zka[D:P], in_=zsrc)
            nc.vector.tensor_copy(out=zkb[0:D], in_=zsrc)

        pair_seq = [
            (2 * pr, 2 * pr + 1) for _ in range(reps) for pr in range(HPC // 2)
        ]
        state, units = prep_pair(*pair_seq[0])
        # Pair 0 has no previous loop to hide its transposes in: emit all K
        # units plus the first Q chunk upfront; its remaining Q units drop
        # into its own qc loops (qc c only reads qt[:, c*512:(c+1)*512]).
        n_upfront = len(units) - 3 * (len(units) // 8)
        for u in units[:n_upfront]:
            u()
        units = units[n_upfront:]
        for pi, (hA, hB) in enumerate(pair_seq):
            for u in units[: max(0, len(units) - 16)]:
                u()  # leftovers beyond one pair's absorption capacity
            units = units[max(0, len(units) - 16) :]
            qt, kts, v1s = state
            # whole-pair output staging: one 512KB store per head at pair
            # end (2KB DRAM chunks) instead of eight 128KB per-qc stores.
            o_heads = [
                op.tile([P, NB, RB, D], fp32, tag=f"o_h{i}", name=f"o_h{i}")
                for i in range(2)
            ]
            next_units = []
            if pi + 1 < len(pair_seq):
                state, next_units = prep_pair(*pair_seq[pi + 1])
            units = units + next_units

            # One global iteration stream per pair: PV lags the scores by
            # 2 iterations ACROSS q-chunk boundaries, so the PV pipeline
            # never drains/refills at the 3 interior boundaries. The
            # epilogue is split: the PSUM-release copies issue right after
            # pv(qc,15); the PE transposes+normalize run 4 iterations later
            # (slot t=5 of the next chunk) so they never block the next
            # scores matmuls on the copy in PE program order.
            o_by_qc = {}
            pts = {}
            ot_by_qc = {}
            NTOT = N_QCHUNKS * T_TILES

            def pv(git):
                pqc, pt_ = divmod(git, T_TILES)
                o_pss = o_by_qc[pqc]
                ptile = pts.pop(git)
                for i in range(2):
                    nc.tensor.matmul(
                        o_pss[i],
                        lhsT=v1s[i][:, pt_],
                        rhs=ptile[:, i * QCHUNK : (i + 1) * QCHUNK],
                        start=(pt_ == 0),
                        stop=(pt_ == T_TILES - 1),
                    )

            def ep_copies(qc):
                o_pss = o_by_qc[qc]
                obs = []
                for i in range(2):
                    ot_sb = otp.tile([VE, QCHUNK], fp32, tag="ot_sb")
                    nc.vector.tensor_copy(out=ot_sb, in_=o_pss[i])
                    obs.append(ot_sb)
                ot_by_qc[qc] = obs
                del o_by_qc[qc]

            def ep_late(qc):
                nn = (4 * qc) // RB
                r0 = (4 * qc) % RB
                for i in range(2):
                    ot_sb = ot_by_qc[qc][i]
                    tef = ps_t.tile([P, 4, P], fp32, tag="qk_t", name="tef")
                    te = tef[:, :, 0:VE]
                    for j in range(4):
                        nc.tensor.transpose(
                            te[:, j],
                            ot_sb[:, j * P : (j + 1) * P],
                            identity[0:VE, 0:VE],
                        )
                    rec = rp.tile([P, 4], fp32, tag="rec")
                    nc.vector.reciprocal(out=rec, in_=te[:, :, D])
                    for j in range(4):
                        nc.vector.tensor_scalar_mul(
                            o_heads[i][:, nn, r0 + j],
                            te[:, j, 0:D],
                            rec[:, j : j + 1],
                        )
                del ot_by_qc[qc]

            for git in range(NTOT + 2 + 4):
                if git < NTOT:
                    qc, t = divmod(git, T_TILES)
                    if t == 0:
                        o_by_qc[qc] = (
                            ps_oA.tile([VE, QCHUNK], fp32, tag="o_accA", name="o_psA"),
                            ps_oB.tile([VE, QCHUNK], fp32, tag="o_accB", name="o_psB"),
                        )
                    # absorb pending transpose units in PE slack cycles
                    if t % 4 in (0, 1) and units:
                        units.pop(0)()
                    qsl = slice(qc * QCHUNK, (qc + 1) * QCHUNK)
                    tsl = slice(t * P, (t + 1) * P)
                    s_ps = ps_s.tile([P, 2 * QCHUNK], fp32, tag="scores", name="s_ps")
                    nc.tensor.matmul(
                        s_ps[:, 0:QCHUNK], lhsT=kts[0][:, tsl], rhs=qt[:, qsl],
                        start=True, stop=True,
                    )
                    nc.tensor.matmul(
                        s_ps[:, QCHUNK:], lhsT=kts[1][:, tsl], rhs=qt[:, qsl],
                        start=True, stop=True,
                    )
                lag = git - 2
                if 0 <= lag < NTOT:
                    pv(lag)
                    if lag % T_TILES == T_TILES - 1:
                        ep_copies(lag // T_TILES)
                if git >= 21 and (git - 21) % T_TILES == 0:
                    qe = (git - 21) // T_TILES
                    if qe < N_QCHUNKS:
                        ep_late(qe)
                if git < NTOT:
                    pt = ptp.tile([P, 2 * QCHUNK], bf16, tag="pt", name="pt")
                    pts[git] = pt
                    if t % 16 in sch:
                        # VectorE fast-exp (Schraudolph, bf16 flavor):
                        # s*EXPA16+EXPB16 -> int16 bits of bf16(~exp(0.5 s))
                        nc.vector.tensor_scalar(
                            out=pt.bitcast(i16),
                            in0=s_ps,
                            scalar1=EXPA16,
                            scalar2=EXPB16,
                            op0=mybir.AluOpType.mult,
                            op1=mybir.AluOpType.add,
                        )
                    else:
                        nc.scalar.activation(
                            out=pt, in_=s_ps, func=Exp, scale=SCALE
                        )
            for i, hh in enumerate((hA, hB)):
                nc.sync.dma_start(
                    out=out_ext[hh].rearrange("(n p r) d -> p n r d", p=P, r=RB),
                    in_=o_heads[i],
                )
    nc.finalize()
    return nc


def _get_nc(reps=1, sch=SCH):
    key = f"nc{reps}s{sch}"
    if key not in _CACHE:
        _CACHE[key] = _build_nc(reps, sch=sch)
    return _CACHE[key]


def _shard(x):
    x = np.ascontiguousarray(np.asarray(x), dtype=np.float32).reshape(B * H, S, D)
    return [np.ascontiguousarray(x[i * HPC : (i + 1) * HPC]) for i in range(N_CORES)]


def run(q, k, v, trace=False, **kw):
    from concourse.bass_utils import run_bass_kernel_spmd

    qs, ks, vs = _shard(q), _shard(k), _shard(v)
    in_maps = [{"q": qs[i], "k": ks[i], "v": vs[i]} for i in range(N_CORES)]
    res = run_bass_kernel_spmd(
        _get_nc(), in_maps, core_ids=list(range(N_CORES)), trace=trace, **kw
    )
    out = np.concatenate([res.results[i]["out"] for i in range(N_CORES)], axis=0)
    return out.reshape(B, H, S, D), res


def kernel(q, k, v):
    out, _ = run(q, k, v)
    return out

